# revision 26
# baseline (speedup 1.0000x reference)
"""Trainium2 Bass kernel for channel-wise EMA over per-step batch means.

Problem: x [4, 8192, 1024] f32, ema [1, 1024] f32 (initial state).
    m = mean(x, axis=0)                      # [S, D]
    e_s = a*e_{s-1} + (1-a)*m_s              # scan over S
    out = broadcast(e, [4, S, D])

Strategy: tensor-parallel over D (8 cores x 128 channels each). The EMA is a
linear recurrence computed with matmuls against constant decay operators:
  - per group of 4 chunks x 128 steps, one 1MB DMA loads x as
    [k=128, (b=4, c=4, d=128)]; 4 matmuls against LT4 (lower-triangular
    decay / 4) accumulate the within-chunk EMA in PSUM, folding the batch
    mean into the contraction; 4 more matmuls against w4 (the last row of
    the decay operator) produce the chunk-final local states ("lasts").
  - cross-chunk carries follow carry[c] = a^128 * carry[c-1] + lasts[c-1],
    evaluated exactly as 4 tiny fused scalar_tensor_tensor ops per group on
    the vector engine in flat [1, (c,d)] layout.
  - one rank-1 correction matmul (alpha powers x carries) accumulates into
    the group PSUM; one vector-engine evacuation per group, then DMA out.
"""

import numpy as np

ALPHA = 0.99
B, S, D = 4, 8192, 1024
N_CORES = 8
DSH = D // N_CORES        # 128 channels per core
T = 128                   # chunk length (matmul contraction)
G = 4                     # chunks per group
W = G * DSH               # 512 free width
NG = S // (T * G)         # 16 groups
ALPHA_T = float(np.float64(ALPHA) ** T)


def _consts():
    # Output rows are time-REVERSED within each chunk (out row t' holds
    # timestep 127-t'), so each chunk's local-last lands in PSUM row 0
    # (32-aligned, directly readable by the vector engine) and the
    # post-correction row 0 is exactly the next chunk's carry. The host
    # un-reverses with a free numpy reshuffle.
    al = np.float64(ALPHA)
    k = np.arange(T)[:, None]
    tp = np.arange(T)[None, :]
    t = (T - 1) - tp  # timestep held by output row t'
    # LT4R[k, t'] = 0.25*(1-a)*a^(t-k) for k <= t   (lhsT layout [K, M])
    lt4 = np.where(k <= t, 0.25 * (1.0 - al) * al ** (t - k), 0.0).astype(np.float32)
    # aTR[0, t'] = a^(t+1) = a^(128-t')
    at = (al ** (t[0].astype(np.float64) + 1)).astype(np.float32)[None, :]
    return lt4, at


def build_nc():
    import concourse.mybir as mybir
    import concourse.tile as tile
    from concourse import bacc
    from concourse.bass import ts as bts

    FP32 = mybir.dt.float32
    FP32R = mybir.dt.float32r
    MULT = mybir.AluOpType.mult
    ADD = mybir.AluOpType.add

    nc = bacc.Bacc(trn_type="TRN2")
    x_dram = nc.dram_tensor("x", [B, S, DSH], FP32R, kind="ExternalInput")
    e0_dram = nc.dram_tensor("ema", [1, DSH], FP32, kind="ExternalInput")
    out_dram = nc.dram_tensor("out", [S, DSH], FP32, kind="ExternalOutput")

    lt4_np, at_np = _consts()
    lt4_dram = nc.inline_tensor(lt4_np, "lt4c")
    at_dram = nc.inline_tensor(at_np, "atc")

    # DRAM views: s = c*128 + k globally; supergroups batch several groups
    # into one 3-dim DMA [k, c, d]. The final supergroups are single-group so
    # the pipeline tail after the last load stays short.
    SGS = [2] * 7 + [1, 1]
    assert sum(SGS) == NG
    xv = x_dram.rearrange("b (c k) d -> b k c d", k=T)
    ov = out_dram.rearrange("(g c k) d -> g k c d", g=NG, c=G, k=T)

    with tile.TileContext(nc) as tc:
        with (
            tc.tile_pool(name="const", bufs=1) as cpool,
            tc.tile_pool(name="xin", bufs=3) as xpool,
            tc.tile_pool(name="oout", bufs=6) as opool,
            tc.tile_pool(name="cflat", bufs=3) as fpool,
            tc.tile_pool(name="ypsum", bufs=6, space="PSUM") as ypool,
        ):
            lt4 = cpool.tile([T, T], FP32R)
            nc.scalar.dma_start(lt4[:], lt4_dram[:].bitcast(FP32R))
            at = cpool.tile([1, T], FP32R)
            nc.scalar.dma_start(at[:], at_dram[:].bitcast(FP32R))
            e0 = cpool.tile([1, DSH], FP32)
            nc.scalar.dma_start(e0[:], e0_dram[:])

            # per-group state emitted in a software-pipelined order so the
            # tensor engine is never head-of-line blocked by the carry chain
            state = {}

            def emit_load(sg, g0, ng):
                xts = []
                c0 = g0 * G
                for b in range(B):
                    xt = xpool.tile(
                        [T, ng * W], FP32R, name=f"x{sg}b{b}", tag=f"xt{b}"
                    )
                    nc.sync.dma_start(
                        xt.rearrange("k (c d) -> k c d", c=G * ng),
                        xv[b, :, c0 : c0 + G * ng, :],
                    )
                    xts.append(xt)
                for i in range(ng):
                    state[("x", g0 + i)] = (xts, i)

            def emit_front(g):
                xts, i = state.pop(("x", g))
                ypsum = ypool.tile([T, W], FP32, name=f"ypsum{g}", tag="yp")
                for b in range(B):
                    nc.tensor.matmul(
                        ypsum[:],
                        lt4[:],
                        xts[b][:, bts(i, W)],
                        start=(b == 0),
                        stop=(b == B - 1),
                    )
                state[g] = ypsum

            def emit_back(g):
                ypsum = state.pop(g)
                # carries, flat layout [1, (c,d)]:
                #   carry[4g] = post-correction row 0 of the previous group's
                #     last chunk (= E at group entry; e0 for g=0)
                #   carry[4g+c] = a^T * carry[4g+c-1] + pre-correction row 0
                #     of chunk c-1 (its local last)
                cflat = fpool.tile([1, W], FP32R, name=f"cf{g}", tag="cf")
                if g == 0:
                    nc.vector.tensor_copy(cflat[:, 0:DSH], e0[:])
                else:
                    prev_y = state["y_prev"]
                    nc.vector.tensor_copy(
                        cflat[:, 0:DSH], prev_y[0:1, bts(G - 1, DSH)]
                    )
                for c in range(1, G):
                    nc.vector.scalar_tensor_tensor(
                        cflat[:, bts(c, DSH)],
                        cflat[:, bts(c - 1, DSH)],
                        ALPHA_T,
                        ypsum[0:1, bts(c - 1, DSH)],
                        MULT,
                        ADD,
                    )
                state["y_prev"] = ypsum

                # correction: ypsum[t, (c,d)] += a^(t+1) * carry[c, d]
                nc.tensor.matmul(
                    ypsum[:],
                    at[:],
                    cflat[:],
                    start=False,
                    stop=True,
                    skip_group_check=True,
                )
                out_sb = opool.tile([T, W], FP32, name=f"os{g}", tag="os")
                nc.vector.tensor_copy(out_sb[:], ypsum[:])
                nc.scalar.dma_start(
                    ov[g], out_sb.rearrange("k (c d) -> k c d", c=G)
                )

            sg_start = {}
            g0 = 0
            for sg, ng in enumerate(SGS):
                sg_start[g0] = (sg, ng)
                g0 += ng
            for g in range(NG):
                if g in sg_start:
                    sg, ng = sg_start[g]
                    emit_load(sg, g, ng)
                emit_front(g)
                if g >= 1:
                    emit_back(g - 1)
            emit_back(NG - 1)

    nc.compile()
    return nc


_NC_CACHE = None


def _get_nc():
    global _NC_CACHE
    if _NC_CACHE is None:
        _NC_CACHE = build_nc()
    return _NC_CACHE


def run_device(x: np.ndarray, ema: np.ndarray, **kwargs):
    """Run on the 8 NeuronCores; returns (es [S, D], BassKernelResults)."""
    from concourse.bass_utils import run_bass_kernel_spmd

    x = np.ascontiguousarray(x, dtype=np.float32)
    ema = np.ascontiguousarray(ema, dtype=np.float32)
    nc = _get_nc()

    in_maps = []
    for core in range(N_CORES):
        sl = slice(core * DSH, (core + 1) * DSH)
        in_maps.append(
            {
                "x": np.ascontiguousarray(x[:, :, sl]),
                "ema": np.ascontiguousarray(ema[:, sl]),
            }
        )
    res = run_bass_kernel_spmd(nc, in_maps, core_ids=list(range(N_CORES)), **kwargs)
    # device output rows are time-reversed within each 128-step chunk
    es = np.concatenate(
        [
            res.results[i]["out"]
            .reshape(S // T, T, DSH)[:, ::-1, :]
            .reshape(S, DSH)
            for i in range(N_CORES)
        ],
        axis=1,
    )
    return es, res


def kernel(x: np.ndarray, ema: np.ndarray) -> np.ndarray:
    es, _ = run_device(x, ema)
    return np.ascontiguousarray(np.broadcast_to(es[None], (B, S, D)))
